# revision 27
# baseline (speedup 1.0000x reference)
"""GAT (graph attention) kernel for 8 trn2 NeuronCores.

Strategy (dst-sharded, fully data-parallel, no collectives):
  - Nodes are sharded by destination range: core d owns nodes
    [d*N/8, (d+1)*N/8).  Edges are routed (on host) to the core owning
    their destination.  Self loops are slot 0 of every node.
  - The host pre-gathers x columns into edge-slot order (fp16), so the
    device sees, per destination tile of 128 nodes, D slot-blocks of
    [128 features x 128 slots].  Slot p of block j belongs to dst p.
    Padding slots get a "poison" column engineered (8x8 solve on the
    host) so that a_src = -1000 for every head -> their softmax weight
    underflows to exactly 0 and they drop out of num and den.
  - PE pass A: per block, a thin matmul computes a_src[slot]; a second
    accumulating matmul adds a_dst[dst] -> z sits in PSUM, fp32.
  - ACT: ex = exp(leaky(z) - 3) = max(exp(z-3), exp(0.2 z - 3)): two
    Exp activations (one shared table, no table ping-pong) + a DVE max
    that lands in the last 8 columns of the [P, D, 72] m-buffer.  The
    -3 shift cancels in the softmax and bounds the fp16 sums.
  - PE pass B per chunk: h[slot, 64] -> PSUM; ACT evacuates to fp16
    SBUF in c-major [d, c, h] order (free via strided APs); DVE
    m = ex * h runs at 2x with ex broadcast on the middle dim.
  - DVE: two levels of fp16 tree adds + one tensor_reduce tail reduce
    [P, D, 72] -> [P, 72] = [num_raw | den].
  - DVE: out = num_raw * recip(den); POOL adds bias; outputs are
    written fp16 in [128, TPC*64] c-major layout so each output DMA
    line is 512B-contiguous.
"""

import sys

sys.path.insert(0, "/opt/trn_rl_repo")

from contextlib import ExitStack

import numpy as np

import concourse.bacc as bacc
import concourse.bass as bass
import concourse.tile as tile
from concourse import mybir
from concourse.bass_utils import run_bass_kernel_spmd

P = 128
F = 128
HEADS = 8
OUT_C = 8
HC = HEADS * OUT_C  # 64
K = HC + HEADS  # 72: [m | ex] row width
NEG_SLOPE = 0.2
N_CORES = 8
CHUNK = 12  # h-blocks per PSUM chunk (12*128*4B = 3 banks; x2 bufs + a-tile)

f32 = mybir.dt.float32
f16 = mybir.dt.float16
F16 = np.float16
EXP_SHIFT = -3.0  # exp(z-3): cancels in softmax, bounds ex for fp16 sums


def _poison_row(W, att_src):
    """x-row p with p . wsrc_h = -1000 for all heads h (so pad slots get
    softmax weight exp(<=-200) == 0), scaled to stay well inside fp16."""
    W64 = np.asarray(W, np.float64).reshape(F, HEADS, OUT_C)
    a = np.asarray(att_src, np.float64).reshape(HEADS, OUT_C)
    Ws = np.einsum("fhc,hc->fh", W64, a)  # [F, H]
    G = Ws.T @ Ws + 1e-9 * np.eye(HEADS)
    p = -Ws @ np.linalg.solve(G, np.full(HEADS, 1000.0))
    amax = np.abs(p).max()
    if amax > 2.0e4:  # keep fp16-representable; z_pad stays <= -300
        p *= 2.0e4 / amax
    return p.astype(np.float32)


def host_prep(x, edge_index, W, att_src, n_cores=N_CORES):
    """Route edges, degree-sort nodes, pre-gather x into slot order."""
    x = np.ascontiguousarray(np.asarray(x, dtype=np.float32))
    N = x.shape[0]
    assert x.shape[1] == F
    ei = np.asarray(edge_index)
    src = ei[0].astype(np.int64)
    dst = ei[1].astype(np.int64)

    assert N % n_cores == 0
    NPC = N // n_cores
    TPC = -(-NPC // P)
    NPT = TPC * P

    deg = np.bincount(dst, minlength=N).astype(np.int64)
    order = np.argsort(dst, kind="stable")
    src_sorted = src[order].astype(np.int64)
    rowptr = np.zeros(N + 1, np.int64)
    rowptr[1:] = np.cumsum(deg)

    perms = np.full((n_cores, NPT), -1, np.int64)
    for d in range(n_cores):
        nodes = np.arange(d * NPC, (d + 1) * NPC)
        p = nodes[np.argsort(-deg[nodes], kind="stable")]
        perms[d, :NPC] = p

    degp = np.where(perms >= 0, deg[np.clip(perms, 0, N - 1)], 0)
    # slots per tile: max (deg+1) over the tile, padded to a multiple of 2
    D_t = degp.reshape(n_cores, TPC, P).max(axis=2).max(axis=0) + 1
    D_t = ((D_t + 1) // 2) * 2
    D_t = D_t.astype(np.int64)
    TOT = int(D_t.sum()) * P
    Dmax = int(D_t.max())

    SENT = N  # poison row of x_pad (zero softmax weight on device)
    xpadT = np.zeros((F, N + 1), F16)
    xpadT[:, :N] = x.T.astype(F16)
    xpadT[:, N] = _poison_row(W, att_src).astype(F16)

    cols = np.arange(Dmax)[None, :]
    xs_all = []
    for d in range(n_cores):
        p = perms[d]
        valid = p >= 0
        pc = np.clip(p, 0, N - 1)
        g = np.where(valid, deg[pc], 0)[:, None]
        take = rowptr[pc][:, None] + (cols - 1)
        mask = (cols >= 1) & ((cols - 1) < g)
        arr = np.where(
            mask, src_sorted[np.clip(take, 0, max(len(src_sorted) - 1, 0))], SENT
        )
        arr[:, 0] = np.where(valid, pc, 0)  # self slot (junk for dummies)
        # d-major slot order per tile: block j's column p belongs to dst p
        big = np.concatenate(
            [arr[t * P : (t + 1) * P, : D_t[t]].T.ravel() for t in range(TPC)]
        )
        assert big.shape[0] == TOT
        xs_all.append(np.ascontiguousarray(xpadT[:, big]))

    return dict(
        N=N, NPC=NPC, TPC=TPC, NPT=NPT, TOT=TOT, Dmax=Dmax,
        D_t=D_t, perms=perms, xs=xs_all,
    )


def build_program(TOT, NPT, Dmax, D_t, n_cores=N_CORES):
    TPC = len(D_t)
    nc = bacc.Bacc(
        "TRN2", target_bir_lowering=False, debug=False, num_devices=n_cores
    )
    xs_d = nc.dram_tensor("xs", [F, TOT], f16, kind="ExternalInput")
    w_d = nc.dram_tensor("w", [F, HC], f32, kind="ExternalInput")
    asrc_d = nc.dram_tensor("att_src", [1, HC], f32, kind="ExternalInput")
    adst_d = nc.dram_tensor("att_dst", [1, HC], f32, kind="ExternalInput")
    bias_d = nc.dram_tensor("bias", [1, HC], f16, kind="ExternalInput")
    out_d = nc.dram_tensor("out", [P, TPC * HC], f16, kind="ExternalOutput")

    Add = mybir.AluOpType.add
    Mult = mybir.AluOpType.mult
    X = mybir.AxisListType.X
    Exp = mybir.ActivationFunctionType.Exp
    Copy = mybir.ActivationFunctionType.Copy

    with tile.TileContext(nc) as tc, ExitStack() as ctx:
        ctx.enter_context(
            nc.allow_low_precision(reason="fp16 partial sums; gate is 2e-2")
        )
        wp = ctx.enter_context(tc.tile_pool(name="wp", bufs=1))

        # --- weights prep (one-time, fp32 then cast to fp16) -----------
        asrc_b = wp.tile([P, HC], f32)
        adst_b = wp.tile([P, HC], f32)
        bias_b = wp.tile([P, HC], f16)
        nc.sync.dma_start(asrc_b[:], asrc_d[:, :].to_broadcast([P, HC]))
        nc.sync.dma_start(adst_b[:], adst_d[:, :].to_broadcast([P, HC]))
        nc.sync.dma_start(bias_b[:], bias_d[:, :].to_broadcast([P, HC]))

        w_sb = wp.tile([P, 80], f32)
        nc.sync.dma_start(w_sb[:, 0:HC], w_d[:, :])
        wtmp = wp.tile([P, HC], f32)
        nc.vector.tensor_tensor(
            out=wtmp[:], in0=w_sb[:, 0:HC], in1=asrc_b[:], op=Mult
        )
        nc.vector.tensor_reduce(
            w_sb[:, 64:72],
            wtmp[:].rearrange("p (h c) -> p h c", c=OUT_C),
            axis=X, op=Add,
        )
        nc.vector.tensor_tensor(
            out=wtmp[:], in0=w_sb[:, 0:HC], in1=adst_b[:], op=Mult
        )
        nc.vector.tensor_reduce(
            w_sb[:, 72:80],
            wtmp[:].rearrange("p (h c) -> p h c", c=OUT_C),
            axis=X, op=Add,
        )
        w_bf = wp.tile([P, 80], f16)  # [W(64) | Wsrc(8) | Wdst(8)]
        nc.vector.tensor_copy(w_bf[:], w_sb[:])
        shift_b = wp.tile([P, 1], f32)
        nc.vector.memset(shift_b[:], EXP_SHIFT)

        # --- per-tile pipeline ----------------------------------------
        xp = ctx.enter_context(tc.tile_pool(name="xp", bufs=6))
        pa = ctx.enter_context(tc.tile_pool(name="pa", bufs=2, space="PSUM"))
        ph = ctx.enter_context(tc.tile_pool(name="ph", bufs=2, space="PSUM"))
        sp = ctx.enter_context(tc.tile_pool(name="sp", bufs=6))
        mp = ctx.enter_context(tc.tile_pool(name="mp", bufs=4))
        op = ctx.enter_context(tc.tile_pool(name="op", bufs=3))

        obuf = None
        off = 0
        for t in range(TPC):
            Dt = int(D_t[t])
            xs = xp.tile([P, Dt * P], f16, tag="xs")
            nc.sync.dma_start(xs[:], xs_d[:, off : off + Dt * P])

            # pass A: z = a_src[slot] + a_dst[dst] in PSUM
            ps_a = pa.tile([P, Dt * HEADS], f32, tag="ps_a")
            for j in range(Dt):
                nc.tensor.matmul(
                    out=ps_a[:, j * HEADS : (j + 1) * HEADS],
                    lhsT=xs[:, j * P : (j + 1) * P],
                    rhs=w_bf[:, 64:72],
                    start=True, stop=False,
                )
                nc.tensor.matmul(
                    out=ps_a[:, j * HEADS : (j + 1) * HEADS],
                    lhsT=xs[:, 0:P],
                    rhs=w_bf[:, 72:80],
                    start=False, stop=True,
                )

            # ex = exp(leaky(z) - 3) = max(exp(z-3), exp(0.2 z - 3)),
            # written into msb[:, :, 64:72] so den reduces in the tree
            msb = mp.tile([P, Dt * K], f16, tag="msb")
            ex1 = sp.tile([P, Dt * HEADS], f16, tag="ex1")
            nc.scalar.activation(ex1[:], ps_a[:], Exp, bias=shift_b[:, 0:1])
            ex2 = sp.tile([P, Dt * HEADS], f16, tag="ex2")
            nc.scalar.activation(
                ex2[:], ps_a[:], Exp, bias=shift_b[:, 0:1], scale=NEG_SLOPE
            )
            nc.vector.tensor_tensor(
                out=msb[:].rearrange("p (d k) -> p d k", k=K)[:, :, HC:K],
                in0=ex1[:].rearrange("p (d h) -> p d h", h=HEADS),
                in1=ex2[:].rearrange("p (d h) -> p d h", h=HEADS),
                op=mybir.AluOpType.max,
            )

            # pass B: h -> PSUM per chunk; ACT evacuates to fp16 SBUF in
            # c-major [d, c, h] order, so the m multiply broadcasts ex on
            # the MIDDLE dim and runs at 2x on DVE.
            hsb = mp.tile([P, Dt * HC], f16, tag="hsb")
            for c0 in range(0, Dt, CHUNK):
                nblk = min(CHUNK, Dt - c0)
                ps_h = ph.tile([P, CHUNK * P], f32, tag="ps_h")
                for jr in range(nblk):
                    j = c0 + jr
                    nc.tensor.matmul(
                        out=ps_h[:, jr * P : jr * P + HC],
                        lhsT=xs[:, j * P : (j + 1) * P],
                        rhs=w_bf[:, 0:HC],
                        start=True, stop=True,
                    )
                nc.scalar.activation(
                    hsb[:, c0 * HC : (c0 + nblk) * HC]
                    .rearrange("p (d c h) -> p d c h", c=OUT_C, h=HEADS),
                    ps_h[:, 0 : nblk * P]
                    .rearrange("p (d f) -> p d f", f=P)[:, :, 0:HC]
                    .rearrange("p d (h c) -> p d c h", c=OUT_C),
                    Copy,
                )
                nc.vector.tensor_tensor(
                    out=msb[:, c0 * K : (c0 + nblk) * K]
                    .rearrange("p (d k) -> p d k", k=K)[:, :, 0:HC]
                    .rearrange("p d (c h) -> p d c h", h=HEADS),
                    in0=hsb[:, c0 * HC : (c0 + nblk) * HC]
                    .rearrange("p (d c h) -> p d c h", c=OUT_C, h=HEADS),
                    in1=msb[:]
                    .rearrange("p (d k) -> p d k", k=K)[:, c0 : c0 + nblk, HC:K]
                    .unsqueeze(2)
                    .to_broadcast([P, nblk, OUT_C, HEADS]),
                    op=Mult,
                )

            # reduce msb [P, D, 72] -> red [P, 72] = [num_raw | den]:
            # two levels of 2x tree adds, then one tensor_reduce tail
            msb2 = mp.tile([P, (Dmax // 2 + 1) * K], f16, tag="msb2")
            cur, src_buf, level = Dt, msb, 0
            while cur > 1 and level < 2:
                if cur % 2 == 1:
                    nc.vector.tensor_tensor(
                        out=src_buf[:, 0:K],
                        in0=src_buf[:, 0:K],
                        in1=src_buf[:, (cur - 1) * K : cur * K],
                        op=Add,
                    )
                    cur -= 1
                h = cur // 2
                dst_buf = msb2 if src_buf is msb else msb
                nc.vector.tensor_tensor(
                    out=dst_buf[:, 0 : h * K],
                    in0=src_buf[:, 0 : h * K],
                    in1=src_buf[:, h * K : 2 * h * K],
                    op=Add,
                )
                cur, src_buf, level = h, dst_buf, level + 1
            if cur > 1:
                red = sp.tile([P, K], f16, tag="red")
                nc.vector.tensor_reduce(
                    red[:],
                    src_buf[:, 0 : cur * K].rearrange("p (d k) -> p k d", k=K),
                    axis=X, op=Add,
                )
            else:
                red = src_buf

            # out = num_raw * recip(den)
            den = sp.tile([P, HEADS], f32, tag="den")
            nc.vector.tensor_copy(den[:], red[:, HC:K])
            rden = sp.tile([P, HEADS], f16, tag="rden")
            nc.vector.reciprocal(rden[:], den[:])
            ot = sp.tile([P, HC], f16, tag="ot")
            nc.gpsimd.tensor_tensor(
                out=ot[:].rearrange("p (c h) -> p c h", h=HEADS),
                in0=red[:, 0:HC].rearrange("p (c h) -> p c h", h=HEADS),
                in1=rden[:].unsqueeze(1).to_broadcast([P, OUT_C, HEADS]),
                op=Mult,
            )

            # bias on POOL into the 4-tile output buffer
            if t % 4 == 0:
                obuf = op.tile([P, 4 * HC], f16, tag="obuf")
                ot0 = t
            nc.gpsimd.tensor_tensor(
                out=obuf[:, (t - ot0) * HC : (t - ot0 + 1) * HC],
                in0=ot[:], in1=bias_b[:], op=Add,
            )
            if t - ot0 == 3 or t == TPC - 1:
                nc.sync.dma_start(
                    out_d[:, ot0 * HC : (t + 1) * HC],
                    obuf[:, 0 : (t - ot0 + 1) * HC],
                )
            off += Dt * P

    nc.compile()
    return nc


def make_in_maps(prep, W, att_src, att_dst, bias, n_cores=N_CORES):
    W = np.ascontiguousarray(np.asarray(W, np.float32))
    asrc = np.asarray(att_src, np.float32).reshape(1, HC)
    adst = np.asarray(att_dst, np.float32).reshape(1, HC)
    # bias in c-major [c, h] order to match the device-side layout
    b = np.ascontiguousarray(
        np.asarray(bias, np.float32).reshape(HEADS, OUT_C).T
    ).reshape(1, HC).astype(F16)
    return [
        {
            "xs": prep["xs"][d],
            "w": W,
            "att_src": asrc,
            "att_dst": adst,
            "bias": b,
        }
        for d in range(n_cores)
    ]


def unpermute(prep, core_outs, n_cores=N_CORES):
    N, TPC = prep["N"], prep["TPC"]
    full = np.zeros((N, HC), np.float32)
    for d in range(n_cores):
        res = np.asarray(core_outs[d]).astype(np.float32)
        # [P, TPC, c, h] -> [node, (h c)]
        res = (
            res.reshape(P, TPC, OUT_C, HEADS)
            .transpose(1, 0, 3, 2)
            .reshape(-1, HC)
        )
        p = prep["perms"][d]
        v = p >= 0
        full[p[v]] = res[v]
    return full


def kernel(x, edge_index, W, att_src, att_dst, bias):
    prep = host_prep(x, edge_index, W, att_src)
    nc = build_program(prep["TOT"], prep["NPT"], prep["Dmax"], prep["D_t"])
    in_maps = make_in_maps(prep, W, att_src, att_dst, bias)
    res = run_bass_kernel_spmd(nc, in_maps, core_ids=list(range(N_CORES)))
    return unpermute(prep, [r["out"] for r in res.results])


# revision 28
# speedup vs baseline: 1.0549x; 1.0549x over previous
"""GAT (graph attention) kernel for 8 trn2 NeuronCores.

Strategy (dst-sharded, fully data-parallel, no collectives):
  - Nodes are sharded by destination range: core d owns nodes
    [d*N/8, (d+1)*N/8).  Edges are routed (on host) to the core owning
    their destination.  Self loops are slot 0 of every node.
  - The host pre-gathers x columns into edge-slot order (fp16), so the
    device sees, per destination tile of 128 nodes, D slot-blocks of
    [128 features x 128 slots].  Slot p of block j belongs to dst p.
    Padding slots get a "poison" column engineered (8x8 solve on the
    host) so that a_src = -1000 for every head -> their softmax weight
    underflows to exactly 0 and they drop out of num and den.
  - PE pass A: per block, a thin matmul computes a_src[slot]; a second
    accumulating matmul adds a_dst[dst] -> z sits in PSUM, fp32.
  - ACT: ex = exp(leaky(z) - 3) = max(exp(z-3), exp(0.2 z - 3)): two
    Exp activations (one shared table, no table ping-pong) + a DVE max
    that lands in the last 8 columns of the [P, D, 72] m-buffer.  The
    -3 shift cancels in the softmax and bounds the fp16 sums.
  - PE pass B per chunk: h[slot, 64] -> PSUM; ACT evacuates to fp16
    SBUF in c-major [d, c, h] order (free via strided APs); DVE
    m = ex * h runs at 2x with ex broadcast on the middle dim.
  - DVE: two levels of fp16 tree adds + one tensor_reduce tail reduce
    [P, D, 72] -> [P, 72] = [num_raw | den].
  - DVE: out = num_raw * recip(den); POOL adds bias; outputs are
    written fp16 in [128, TPC*64] c-major layout so each output DMA
    line is 512B-contiguous.
"""

import sys

sys.path.insert(0, "/opt/trn_rl_repo")

from contextlib import ExitStack

import numpy as np

import concourse.bacc as bacc
import concourse.bass as bass
import concourse.tile as tile
from concourse import mybir
from concourse.bass_utils import run_bass_kernel_spmd

P = 128
F = 128
HEADS = 8
OUT_C = 8
HC = HEADS * OUT_C  # 64
K = HC + HEADS  # 72: [m | ex] row width
NEG_SLOPE = 0.2
N_CORES = 8
CHUNK = 12  # h-blocks per PSUM chunk (12*128*4B = 3 banks; x2 bufs + a-tile)

f32 = mybir.dt.float32
f16 = mybir.dt.float16
F16 = np.float16
EXP_SHIFT = -3.0  # exp(z-3): cancels in softmax, bounds ex for fp16 sums


def _poison_row(W, att_src):
    """x-row p with p . wsrc_h = -1000 for all heads h (so pad slots get
    softmax weight exp(<=-200) == 0), scaled to stay well inside fp16."""
    W64 = np.asarray(W, np.float64).reshape(F, HEADS, OUT_C)
    a = np.asarray(att_src, np.float64).reshape(HEADS, OUT_C)
    Ws = np.einsum("fhc,hc->fh", W64, a)  # [F, H]
    G = Ws.T @ Ws + 1e-9 * np.eye(HEADS)
    p = -Ws @ np.linalg.solve(G, np.full(HEADS, 1000.0))
    amax = np.abs(p).max()
    if amax > 2.0e4:  # keep fp16-representable; z_pad stays <= -300
        p *= 2.0e4 / amax
    return p.astype(np.float32)


def host_prep(x, edge_index, W, att_src, n_cores=N_CORES):
    """Route edges, degree-sort nodes, pre-gather x into slot order."""
    x = np.ascontiguousarray(np.asarray(x, dtype=np.float32))
    N = x.shape[0]
    assert x.shape[1] == F
    ei = np.asarray(edge_index)
    src = ei[0].astype(np.int64)
    dst = ei[1].astype(np.int64)

    assert N % n_cores == 0
    NPC = N // n_cores
    TPC = -(-NPC // P)
    NPT = TPC * P

    deg = np.bincount(dst, minlength=N).astype(np.int64)
    order = np.argsort(dst, kind="stable")
    src_sorted = src[order].astype(np.int64)
    rowptr = np.zeros(N + 1, np.int64)
    rowptr[1:] = np.cumsum(deg)

    perms = np.full((n_cores, NPT), -1, np.int64)
    for d in range(n_cores):
        nodes = np.arange(d * NPC, (d + 1) * NPC)
        p = nodes[np.argsort(-deg[nodes], kind="stable")]
        perms[d, :NPC] = p

    degp = np.where(perms >= 0, deg[np.clip(perms, 0, N - 1)], 0)
    # slots per tile: max (deg+1) over the tile, padded to a multiple of 2
    D_t = degp.reshape(n_cores, TPC, P).max(axis=2).max(axis=0) + 1
    D_t = ((D_t + 1) // 2) * 2
    D_t = D_t.astype(np.int64)
    TOT = int(D_t.sum()) * P
    Dmax = int(D_t.max())

    SENT = N  # poison row of x_pad (zero softmax weight on device)
    xpadT = np.zeros((F, N + 1), F16)
    xpadT[:, :N] = x.T.astype(F16)
    xpadT[:, N] = _poison_row(W, att_src).astype(F16)

    cols = np.arange(Dmax)[None, :]
    xs_all = []
    for d in range(n_cores):
        p = perms[d]
        valid = p >= 0
        pc = np.clip(p, 0, N - 1)
        g = np.where(valid, deg[pc], 0)[:, None]
        take = rowptr[pc][:, None] + (cols - 1)
        mask = (cols >= 1) & ((cols - 1) < g)
        arr = np.where(
            mask, src_sorted[np.clip(take, 0, max(len(src_sorted) - 1, 0))], SENT
        )
        arr[:, 0] = np.where(valid, pc, 0)  # self slot (junk for dummies)
        # d-major slot order per tile: block j's column p belongs to dst p
        big = np.concatenate(
            [arr[t * P : (t + 1) * P, : D_t[t]].T.ravel() for t in range(TPC)]
        )
        assert big.shape[0] == TOT
        xs_all.append(np.ascontiguousarray(xpadT[:, big]))

    return dict(
        N=N, NPC=NPC, TPC=TPC, NPT=NPT, TOT=TOT, Dmax=Dmax,
        D_t=D_t, perms=perms, xs=xs_all,
    )


def build_program(TOT, NPT, Dmax, D_t, n_cores=N_CORES):
    TPC = len(D_t)
    nc = bacc.Bacc(
        "TRN2", target_bir_lowering=False, debug=False, num_devices=n_cores
    )
    xs_d = nc.dram_tensor("xs", [F, TOT], f16, kind="ExternalInput")
    w_d = nc.dram_tensor("w", [F, HC], f32, kind="ExternalInput")
    asrc_d = nc.dram_tensor("att_src", [1, HC], f32, kind="ExternalInput")
    adst_d = nc.dram_tensor("att_dst", [1, HC], f32, kind="ExternalInput")
    bias_d = nc.dram_tensor("bias", [1, HC], f16, kind="ExternalInput")
    out_d = nc.dram_tensor("out", [P, TPC * HC], f16, kind="ExternalOutput")

    Add = mybir.AluOpType.add
    Mult = mybir.AluOpType.mult
    X = mybir.AxisListType.X
    Exp = mybir.ActivationFunctionType.Exp
    Prelu = mybir.ActivationFunctionType.Prelu
    Copy = mybir.ActivationFunctionType.Copy

    with tile.TileContext(nc) as tc, ExitStack() as ctx:
        ctx.enter_context(
            nc.allow_low_precision(reason="fp16 partial sums; gate is 2e-2")
        )
        wp = ctx.enter_context(tc.tile_pool(name="wp", bufs=1))

        # --- weights prep (one-time, fp32 then cast to fp16) -----------
        asrc_b = wp.tile([P, HC], f32)
        adst_b = wp.tile([P, HC], f32)
        bias_b = wp.tile([P, HC], f16)
        nc.sync.dma_start(asrc_b[:], asrc_d[:, :].to_broadcast([P, HC]))
        nc.sync.dma_start(adst_b[:], adst_d[:, :].to_broadcast([P, HC]))
        nc.sync.dma_start(bias_b[:], bias_d[:, :].to_broadcast([P, HC]))

        w_sb = wp.tile([P, 80], f32)
        nc.sync.dma_start(w_sb[:, 0:HC], w_d[:, :])
        wtmp = wp.tile([P, HC], f32)
        nc.vector.tensor_tensor(
            out=wtmp[:], in0=w_sb[:, 0:HC], in1=asrc_b[:], op=Mult
        )
        nc.vector.tensor_reduce(
            w_sb[:, 64:72],
            wtmp[:].rearrange("p (h c) -> p h c", c=OUT_C),
            axis=X, op=Add,
        )
        nc.vector.tensor_tensor(
            out=wtmp[:], in0=w_sb[:, 0:HC], in1=adst_b[:], op=Mult
        )
        nc.vector.tensor_reduce(
            w_sb[:, 72:80],
            wtmp[:].rearrange("p (h c) -> p h c", c=OUT_C),
            axis=X, op=Add,
        )
        w_bf = wp.tile([P, 80], f16)  # [W(64) | Wsrc(8) | Wdst(8)]
        nc.vector.tensor_copy(w_bf[:], w_sb[:])
        shift_b = wp.tile([P, 1], f32)
        nc.vector.memset(shift_b[:], EXP_SHIFT)

        # --- per-tile pipeline ----------------------------------------
        xp = ctx.enter_context(tc.tile_pool(name="xp", bufs=6))
        pa = ctx.enter_context(tc.tile_pool(name="pa", bufs=2, space="PSUM"))
        ph = ctx.enter_context(tc.tile_pool(name="ph", bufs=2, space="PSUM"))
        sp = ctx.enter_context(tc.tile_pool(name="sp", bufs=6))
        mp = ctx.enter_context(tc.tile_pool(name="mp", bufs=4))
        op = ctx.enter_context(tc.tile_pool(name="op", bufs=3))

        obuf = None
        off = 0
        for t in range(TPC):
            Dt = int(D_t[t])
            xs = xp.tile([P, Dt * P], f16, tag="xs")
            nc.sync.dma_start(xs[:], xs_d[:, off : off + Dt * P])

            # pass A: z = a_src[slot] + a_dst[dst] in PSUM
            ps_a = pa.tile([P, Dt * HEADS], f32, tag="ps_a")
            for j in range(Dt):
                nc.tensor.matmul(
                    out=ps_a[:, j * HEADS : (j + 1) * HEADS],
                    lhsT=xs[:, j * P : (j + 1) * P],
                    rhs=w_bf[:, 64:72],
                    start=True, stop=False,
                )
                nc.tensor.matmul(
                    out=ps_a[:, j * HEADS : (j + 1) * HEADS],
                    lhsT=xs[:, 0:P],
                    rhs=w_bf[:, 72:80],
                    start=False, stop=True,
                )

            # ex = exp(prelu(z) - 3), written into msb[:, :, 64:72] so
            # den reduces in the same tree.  Prelu (= leaky relu) shares
            # the 'exp_and_others' ACT table with Exp and Copy: no ATL
            # ping-pong.
            msb = mp.tile([P, Dt * K], f16, tag="msb")
            zl = sp.tile([P, Dt * HEADS], f16, tag="zl")
            nc.scalar.activation(zl[:], ps_a[:], Prelu, alpha=NEG_SLOPE)
            nc.scalar.activation(
                msb[:].rearrange("p (d k) -> p d k", k=K)[:, :, HC:K],
                zl[:].rearrange("p (d h) -> p d h", h=HEADS),
                Exp, bias=shift_b[:, 0:1],
            )

            # pass B: h -> PSUM per chunk; ACT evacuates to fp16 SBUF in
            # c-major [d, c, h] order, so the m multiply broadcasts ex on
            # the MIDDLE dim and runs at 2x on DVE.
            hsb = mp.tile([P, Dt * HC], f16, tag="hsb")
            for c0 in range(0, Dt, CHUNK):
                nblk = min(CHUNK, Dt - c0)
                ps_h = ph.tile([P, CHUNK * P], f32, tag="ps_h")
                for jr in range(nblk):
                    j = c0 + jr
                    nc.tensor.matmul(
                        out=ps_h[:, jr * P : jr * P + HC],
                        lhsT=xs[:, j * P : (j + 1) * P],
                        rhs=w_bf[:, 0:HC],
                        start=True, stop=True,
                    )
                nc.scalar.activation(
                    hsb[:, c0 * HC : (c0 + nblk) * HC]
                    .rearrange("p (d c h) -> p d c h", c=OUT_C, h=HEADS),
                    ps_h[:, 0 : nblk * P]
                    .rearrange("p (d f) -> p d f", f=P)[:, :, 0:HC]
                    .rearrange("p d (h c) -> p d c h", c=OUT_C),
                    Copy,
                )
                nc.vector.tensor_tensor(
                    out=msb[:, c0 * K : (c0 + nblk) * K]
                    .rearrange("p (d k) -> p d k", k=K)[:, :, 0:HC]
                    .rearrange("p d (c h) -> p d c h", h=HEADS),
                    in0=hsb[:, c0 * HC : (c0 + nblk) * HC]
                    .rearrange("p (d c h) -> p d c h", c=OUT_C, h=HEADS),
                    in1=msb[:]
                    .rearrange("p (d k) -> p d k", k=K)[:, c0 : c0 + nblk, HC:K]
                    .unsqueeze(2)
                    .to_broadcast([P, nblk, OUT_C, HEADS]),
                    op=Mult,
                )

            # reduce msb [P, D, 72] -> red [P, 72] = [num_raw | den]:
            # two levels of 2x tree adds, then one tensor_reduce tail
            msb2 = mp.tile([P, (Dmax // 2 + 1) * K], f16, tag="msb2")
            cur, src_buf, level = Dt, msb, 0
            while cur > 1 and level < 2:
                if cur % 2 == 1:
                    nc.vector.tensor_tensor(
                        out=src_buf[:, 0:K],
                        in0=src_buf[:, 0:K],
                        in1=src_buf[:, (cur - 1) * K : cur * K],
                        op=Add,
                    )
                    cur -= 1
                h = cur // 2
                dst_buf = msb2 if src_buf is msb else msb
                nc.vector.tensor_tensor(
                    out=dst_buf[:, 0 : h * K],
                    in0=src_buf[:, 0 : h * K],
                    in1=src_buf[:, h * K : 2 * h * K],
                    op=Add,
                )
                cur, src_buf, level = h, dst_buf, level + 1
            if cur > 1:
                red = sp.tile([P, K], f16, tag="red")
                nc.vector.tensor_reduce(
                    red[:],
                    src_buf[:, 0 : cur * K].rearrange("p (d k) -> p k d", k=K),
                    axis=X, op=Add,
                )
            else:
                red = src_buf

            # out = num_raw * recip(den)
            den = sp.tile([P, HEADS], f32, tag="den")
            nc.vector.tensor_copy(den[:], red[:, HC:K])
            rden = sp.tile([P, HEADS], f16, tag="rden")
            nc.vector.reciprocal(rden[:], den[:])
            ot = sp.tile([P, HC], f16, tag="ot")
            nc.gpsimd.tensor_tensor(
                out=ot[:].rearrange("p (c h) -> p c h", h=HEADS),
                in0=red[:, 0:HC].rearrange("p (c h) -> p c h", h=HEADS),
                in1=rden[:].unsqueeze(1).to_broadcast([P, OUT_C, HEADS]),
                op=Mult,
            )

            # bias on POOL into the 4-tile output buffer
            if t % 4 == 0:
                obuf = op.tile([P, 4 * HC], f16, tag="obuf")
                ot0 = t
            nc.gpsimd.tensor_tensor(
                out=obuf[:, (t - ot0) * HC : (t - ot0 + 1) * HC],
                in0=ot[:], in1=bias_b[:], op=Add,
            )
            if t - ot0 == 3 or t == TPC - 1:
                nc.sync.dma_start(
                    out_d[:, ot0 * HC : (t + 1) * HC],
                    obuf[:, 0 : (t - ot0 + 1) * HC],
                )
            off += Dt * P

    nc.compile()
    return nc


def make_in_maps(prep, W, att_src, att_dst, bias, n_cores=N_CORES):
    W = np.ascontiguousarray(np.asarray(W, np.float32))
    asrc = np.asarray(att_src, np.float32).reshape(1, HC)
    adst = np.asarray(att_dst, np.float32).reshape(1, HC)
    # bias in c-major [c, h] order to match the device-side layout
    b = np.ascontiguousarray(
        np.asarray(bias, np.float32).reshape(HEADS, OUT_C).T
    ).reshape(1, HC).astype(F16)
    return [
        {
            "xs": prep["xs"][d],
            "w": W,
            "att_src": asrc,
            "att_dst": adst,
            "bias": b,
        }
        for d in range(n_cores)
    ]


def unpermute(prep, core_outs, n_cores=N_CORES):
    N, TPC = prep["N"], prep["TPC"]
    full = np.zeros((N, HC), np.float32)
    for d in range(n_cores):
        res = np.asarray(core_outs[d]).astype(np.float32)
        # [P, TPC, c, h] -> [node, (h c)]
        res = (
            res.reshape(P, TPC, OUT_C, HEADS)
            .transpose(1, 0, 3, 2)
            .reshape(-1, HC)
        )
        p = prep["perms"][d]
        v = p >= 0
        full[p[v]] = res[v]
    return full


def kernel(x, edge_index, W, att_src, att_dst, bias):
    prep = host_prep(x, edge_index, W, att_src)
    nc = build_program(prep["TOT"], prep["NPT"], prep["Dmax"], prep["D_t"])
    in_maps = make_in_maps(prep, W, att_src, att_dst, bias)
    res = run_bass_kernel_spmd(nc, in_maps, core_ids=list(range(N_CORES)))
    return unpermute(prep, [r["out"] for r in res.results])
